# revision 12
# baseline (speedup 1.0000x reference)
"""Binarized 3x3 conv (nn_BiTestConv2d) — TRN2 Bass/Tile kernel.

Reference math:
    bx = sign(x)                                  (forward value of the STE)
    bw = w - mean_o(w);  scale_o = mean|bw|;  binary_w = scale_o * sign(bw)
    y  = conv2d(bx, binary_w, stride 1, pad 1)    (NCHW / OIHW)

Kernel strategy (per core, data-parallel over batch: 32 imgs / 8 cores):
  - weight prep on device: per-out-channel mean & scale (two-stage f32
    reductions), sign -> bf16, PE-transpose, convert to fp8 lhsT layout
    [ci, 2, khw, co] (dim1 = the DoubleRow K-pair index = ci_tile)
  - activations: DMA f32, ACT Sign -> fp8 into a zero-padded [128, 2, 58*58]
    image tile (both channel halves); borders zeroed once per buffer
  - conv: fp8 DoubleRow matmuls, K=256 (2x128 via perf mode), M=C_out tile,
    N=464 = 8 *padded* output rows of 58 (contiguous rhs; junk edge columns
    computed then discarded at drain) -- 9 accumulating matmuls per PSUM tile
  - drain: PSUM[valid 8x56] * scale_o -> SBUF f32 -> DMA out

All matmul operands are exactly +-1 (or 0 padding) in fp8e4; PSUM partial
sums are exact integers <= 2304 in f32, so the conv itself is exact; only
the final scale multiply rounds.
"""

import numpy as np

# Problem shapes (hardcoded; the harness calls kernel() with exactly these).
N_FULL, C, H, W = 32, 256, 56, 56
KH, KW = 3, 3
N_CORES = 8
N_IMG = N_FULL // N_CORES  # images per core
P = 128                    # partitions
CI_T = C // P              # 2 input-channel halves (DoubleRow pair)
CO_T = C // P              # 2 output-channel tiles
HP = H + 2                 # padded rows/cols (58)
BLK = 8                    # output rows per PSUM tile
NB = H // BLK              # 7 row-blocks
NPIX_B = BLK * W           # 448 valid px per block
NRAS_B = BLK * HP          # 464 padded-raster px per block
KFLAT = C * KH * KW        # 2304
IMG_OFF = 0                # image start offset inside each channel plane
PLANE = 3392               # padded plane stride (>= 58*58, 16-byte aligned)


def build_module(
    loop: int = 1,
    prep_in_loop: bool = False,
    hint_pe: bool = False,
    yout_gpsimd: bool = False,
    xraw_bufs: int = 4,
    staggered: bool = False,
    drain_act: bool = False,
    merged_in: bool = False,
    y_f16: bool = False,
    batched_out: bool = False,
):
    import contextlib
    from contextlib import ExitStack

    import concourse.bass as bass  # noqa: F401  (AP helpers)
    import concourse.mybir as mybir
    import concourse.tile as tile
    from concourse import bacc
    from concourse.masks import make_identity

    f32 = mybir.dt.float32
    bf16 = mybir.dt.bfloat16
    f16 = mybir.dt.float16
    fp8 = mybir.dt.float8e4
    y_dt = f16 if y_f16 else f32

    nc = bacc.Bacc("TRN2", target_bir_lowering=False, debug=False)

    x_d = nc.dram_tensor("x", [N_IMG, C, H, W], f32, kind="ExternalInput").ap()
    w_d = nc.dram_tensor("weight", [C, C, KH, KW], f32, kind="ExternalInput").ap()
    y_d = nc.dram_tensor("y", [N_IMG, C, H, W], y_dt, kind="ExternalOutput").ap()

    w_flat = w_d.rearrange("o i kh kw -> o i (kh kw)")  # [256, 256, 9]
    y_flat = y_d.rearrange("n c h w -> n c (h w)")      # [4, 256, 3136]

    with tile.TileContext(nc) as tc, ExitStack() as ctx:
        consts = ctx.enter_context(tc.tile_pool(name="consts", bufs=1))
        wprep = ctx.enter_context(tc.tile_pool(name="wprep", bufs=2))
        xraw_p = ctx.enter_context(tc.tile_pool(name="xraw", bufs=xraw_bufs))
        ysb_p = ctx.enter_context(
            tc.tile_pool(name="ysb", bufs=4 if batched_out else 8)
        )
        psum_p = ctx.enter_context(tc.tile_pool(name="psum", bufs=8, space="PSUM"))

        identity = consts.tile([P, P], bf16)
        make_identity(nc, identity)

        # weight tiles: double-buffered when prep runs inside the loop so
        # iteration i+1's prep writes don't WAR-stall on iteration i's conv
        wconst = ctx.enter_context(
            tc.tile_pool(name="wconst", bufs=2 if prep_in_loop else 1)
        )

        # fixed activation tiles, one per image: [ci(part), j, padded plane]
        xtiles = [
            consts.tile([P, CI_T, PLANE], fp8, name=f"xt{i}") for i in range(N_IMG)
        ]
        for t in xtiles:
            nc.gpsimd.memset(t, 0.0)

        # ---------------- weight prep ----------------
        def weight_prep():
            # DoubleRow lhsT: [ci(part), j=ci_tile, khw, co] fp8, +-1
            wT = wconst.tile([P, CI_T, KH * KW, C], fp8, tag="wT")
            # per-out-channel scale, column per co_tile: [co(part), co_t] f32
            scale_sb = wconst.tile([P, CO_T], f32, tag="scale")
            for co_t in range(CO_T):
              o0 = co_t * P
              w_sb = wprep.tile([P, C, KH * KW], f32, tag="w_sb")
              nc.sync.dma_start(out=w_sb, in_=w_flat[o0 : o0 + P])

              # two-stage mean over (i, khw): sums of 9, then sum of 256
              s1 = wprep.tile([P, C], f32, tag="s1")
              nc.vector.reduce_sum(out=s1, in_=w_sb, axis=mybir.AxisListType.X)
              s2 = wprep.tile([P, 1], f32, tag="s2")
              nc.vector.reduce_sum(out=s2, in_=s1, axis=mybir.AxisListType.X)
              mean = wprep.tile([P, 1], f32, tag="mean")
              nc.scalar.mul(out=mean, in_=s2, mul=1.0 / KFLAT)

              bw = wprep.tile([P, C, KH * KW], f32, tag="bw")
              nc.vector.tensor_scalar_sub(out=bw, in0=w_sb, scalar1=mean)

              a1 = wprep.tile([P, C], f32, tag="a1")
              nc.vector.tensor_reduce(
                  out=a1,
                  in_=bw,
                  axis=mybir.AxisListType.X,
                  op=mybir.AluOpType.add,
                  apply_absolute_value=True,
              )
              a2 = wprep.tile([P, 1], f32, tag="a2")
              nc.vector.reduce_sum(out=a2, in_=a1, axis=mybir.AxisListType.X)
              nc.scalar.mul(out=scale_sb[:, co_t : co_t + 1], in_=a2, mul=1.0 / KFLAT)

              # sign(bw) -> bf16, written permuted to [o(part), khw, i]
              wsign = wprep.tile([P, KH * KW, C], bf16, tag="wsign")
              nc.scalar.sign(out=wsign.rearrange("p khw i -> p i khw"), in_=bw)

              # PE-transpose each [o, 128-i] block -> wT[i, ci_t, khw, o] (fp8)
              for ci_t in range(CI_T):
                  i0 = ci_t * P
                  for khw in range(KH * KW):
                      pt = psum_p.tile([P, P], bf16, tag="acc", name=f"tp_{co_t}_{ci_t}_{khw}")
                      nc.tensor.transpose(pt, wsign[:, khw, i0 : i0 + P], identity)
                      nc.vector.tensor_copy(
                          out=wT[:, ci_t, khw, o0 : o0 + P], in_=pt
                      )
            return wT, scale_sb

        if not prep_in_loop:
            wT, scale_sb = weight_prep()

        # ---------------- conv ----------------
        hints = (mybir.EngineType.PE,) if hint_pe else ()
        loop_cm = (
            tc.For_i(0, loop, 1, hint_engines=hints, staggered_reset=staggered)
            if loop > 1
            else contextlib.nullcontext()
        )
        with loop_cm:
            if prep_in_loop:
                wT, scale_sb = weight_prep()
            for img in range(N_IMG):
                xt = xtiles[img]
                if merged_in:
                    xr = xraw_p.tile([P, CI_T, H, W], f32, tag="xr")
                    nc.sync.dma_start(
                        out=xr,
                        in_=x_d[img].rearrange("(j c) h w -> c j h w", j=CI_T),
                    )
                    interior = xt[:, :, IMG_OFF : IMG_OFF + HP * HP].rearrange(
                        "p j (h w) -> p j h w", w=HP
                    )[:, :, 1 : HP - 1, 1 : HP - 1]
                    nc.scalar.sign(out=interior, in_=xr)
                else:
                    for ci_t in range(CI_T):
                        i0 = ci_t * P
                        xr = xraw_p.tile([P, H, W], f32, tag="xr")
                        nc.sync.dma_start(out=xr, in_=x_d[img, i0 : i0 + P])
                        interior = (
                            xt[:, ci_t, IMG_OFF : IMG_OFF + HP * HP]
                            .rearrange("p (h w) -> p h w", w=HP)[:, 1 : HP - 1, 1 : HP - 1]
                        )
                        nc.scalar.sign(out=interior, in_=xr)

                xt_v = xt[:, :, IMG_OFF : IMG_OFF + HP * HP].rearrange(
                    "p j (h w) -> p j h w", w=HP
                )
                dma_eng = nc.gpsimd if yout_gpsimd else nc.sync
                for co_t in range(CO_T):
                    o0 = co_t * P
                    yimg = (
                        ysb_p.tile(
                            [P, H * W], y_dt, tag="ysb", name=f"yimg_{img}_{co_t}"
                        )
                        if batched_out
                        else None
                    )
                    for b in range(NB):
                        ps = psum_p.tile(
                            [P, BLK, W], f32, tag="acc",
                            name=f"ps_{img}_{co_t}_{b}",
                        )
                        for khw in range(KH * KW):
                            kh, kw = divmod(khw, KW)
                            nc.tensor.matmul(
                                ps,
                                wT[:, :, khw, o0 : o0 + P],
                                xt_v[
                                    :, :,
                                    b * BLK + kh : b * BLK + kh + BLK,
                                    kw : kw + W,
                                ],
                                start=(khw == 0),
                                stop=(khw == KH * KW - 1),
                                perf_mode=mybir.MatmulPerfMode.DoubleRow,
                            )
                        if batched_out:
                            ysb = yimg[:, b * NPIX_B : (b + 1) * NPIX_B]
                        else:
                            ysb = ysb_p.tile(
                                [P, NPIX_B], y_dt, tag="ysb",
                                name=f"ysb_{img}_{co_t}_{b}",
                            )
                        if drain_act:
                            nc.scalar.mul(
                                out=ysb, in_=ps,
                                mul=scale_sb[:, co_t : co_t + 1],
                            )
                        else:
                            nc.vector.tensor_scalar_mul(
                                out=ysb,
                                in0=ps,
                                scalar1=scale_sb[:, co_t : co_t + 1],
                            )
                        if not batched_out:
                            dma_eng.dma_start(
                                out=y_flat[
                                    img, o0 : o0 + P, b * NPIX_B : (b + 1) * NPIX_B
                                ],
                                in_=ysb,
                            )
                    if batched_out:
                        dma_eng.dma_start(
                            out=y_flat[img, o0 : o0 + P], in_=yimg
                        )

    nc.compile()
    return nc


_CACHED_NC = None


def kernel(x: np.ndarray, weight: np.ndarray) -> np.ndarray:
    global _CACHED_NC
    from concourse.bass_utils import run_bass_kernel_spmd

    if _CACHED_NC is None:
        _CACHED_NC = build_module(
            yout_gpsimd=True,
            xraw_bufs=3,
            merged_in=True,
            y_f16=True,
            batched_out=True,
        )
    nc = _CACHED_NC

    x = np.ascontiguousarray(x, dtype=np.float32)
    weight = np.ascontiguousarray(weight, dtype=np.float32)
    in_maps = [
        {"x": x[c * N_IMG : (c + 1) * N_IMG], "weight": weight}
        for c in range(N_CORES)
    ]
    res = run_bass_kernel_spmd(nc, in_maps, core_ids=list(range(N_CORES)))
    return np.concatenate(
        [r["y"].astype(np.float32) for r in res.results], axis=0
    )



# revision 30
# speedup vs baseline: 1.2612x; 1.2612x over previous
"""Binarized 3x3 conv (nn_BiTestConv2d) — TRN2 Bass/Tile kernel.

Reference math:
    bx = sign(x)                                  (forward value of the STE)
    bw = w - mean_o(w);  scale_o = mean|bw|;  binary_w = scale_o * sign(bw)
    y  = conv2d(bx, binary_w, stride 1, pad 1)    (NCHW / OIHW)

Kernel strategy (per core, data-parallel over batch: 32 imgs / 8 cores):
  - weight prep on device: per-out-channel mean & scale (two-stage f32
    reductions), sign -> bf16, PE-transpose, convert to fp8 lhsT layout
    [ci, 2, khw, co] (dim1 = the DoubleRow K-pair index = ci_tile)
  - activations: DMA f32, ACT Sign -> fp8 into a zero-padded [128, 2, 58*58]
    image tile (both channel halves); borders zeroed once per buffer
  - conv: fp8 DoubleRow matmuls, K=256 (2x128 via perf mode), M=C_out tile,
    N=448 = 8 output rows of 56 (strided windows into the padded plane) --
    9 accumulating matmuls per PSUM tile
  - drain: PSUM[8x56] * scale_o -> SBUF f16 -> one DMA per (img, co_tile)
  - f16 output staging halves the output HBM bytes; the host upcasts to
    f32 (values are exact-integer sums times a f32 scale, so f16 rounding
    is ~2^-11 relative -- far inside the 2e-2 gate)

All matmul operands are exactly +-1 (or 0 padding) in fp8e4; PSUM partial
sums are exact integers <= 2304 in f32, so the conv itself is exact; only
the final scale multiply and the f16 store round.

Performance structure (per For_i tick, `unroll` logical iterations):
  - two wT/scale slots alternate so iteration k's weight prep (when
    prep_in_loop) overlaps iteration k-1's conv instead of serializing
  - prep is emitted in two phases: DMA+reductions+sign before the convs
    (SP/DVE/ACT work that overlaps the PE stream) and the PE transposes
    after the convs (so they don't delay conv matmuls in the PE queue)
  - prep PE-transposes use their own small PSUM pool; sharing the conv
    accumulator ring would serialize prep behind conv drains
"""

import numpy as np

# Problem shapes (hardcoded; the harness calls kernel() with exactly these).
N_FULL, C, H, W = 32, 256, 56, 56
KH, KW = 3, 3
N_CORES = 8
N_IMG = N_FULL // N_CORES  # images per core
P = 128                    # partitions
CI_T = C // P              # 2 input-channel halves (DoubleRow pair)
CO_T = C // P              # 2 output-channel tiles
HP = H + 2                 # padded rows/cols (58)
BLK = 8                    # output rows per PSUM tile
NB = H // BLK              # 7 row-blocks
NPIX_B = BLK * W           # 448 valid px per block
KFLAT = C * KH * KW        # 2304
PLANE = 3392               # padded plane stride (>= 58*58, 16-byte aligned)


def build_module(
    loop: int = 1,
    prep_in_loop: bool = False,
    unroll: int = 1,
    hint_pe: bool = False,
    yout_gpsimd: bool = True,
    xraw_bufs: int = 3,
    ysb_bufs: int = 4,
    staggered: bool = True,
    drain_act: bool = False,
    y_f16: bool = True,
    conv_bf16: bool = False,      # diagnostic: bf16 matmuls, no DoubleRow
    split_psum: bool = True,
    prep_pipelined: bool = True,
    prep_dve_t: bool = False,
    prep_copy_split: bool = True,
    prep_t_pair: int = 2,
    psum_w_bufs: int = 2,
    skip_in_dma: bool = False,
    skip_sign: bool = False,
    skip_mm: bool = False,
    skip_drain: bool = False,
    skip_out_dma: bool = False,
):
    import contextlib
    from contextlib import ExitStack

    import concourse.bass as bass  # noqa: F401  (AP helpers)
    import concourse.mybir as mybir
    import concourse.tile as tile
    from concourse import bacc
    from concourse.masks import make_identity

    f32 = mybir.dt.float32
    bf16 = mybir.dt.bfloat16
    f16 = mybir.dt.float16
    fp8 = mybir.dt.float8e4
    y_dt = f16 if y_f16 else f32
    mm_dt = bf16 if conv_bf16 else fp8

    if loop > 1:
        assert loop % unroll == 0, (loop, unroll)
        ticks = loop // unroll
    else:
        ticks = 1
        unroll = 1

    n_slots = 2 if (prep_in_loop and prep_pipelined and loop > 1) else 1

    nc = bacc.Bacc("TRN2", target_bir_lowering=False, debug=False)

    x_d = nc.dram_tensor("x", [N_IMG, C, H, W], f32, kind="ExternalInput").ap()
    w_d = nc.dram_tensor("weight", [C, C, KH, KW], f32, kind="ExternalInput").ap()
    y_d = nc.dram_tensor("y", [N_IMG, C, H, W], y_dt, kind="ExternalOutput").ap()

    w_flat = w_d.rearrange("o i kh kw -> o i (kh kw)")  # [256, 256, 9]
    y_flat = y_d.rearrange("n c h w -> n c (h w)")      # [4, 256, 3136]

    with tile.TileContext(nc) as tc, ExitStack() as ctx:
        consts = ctx.enter_context(tc.tile_pool(name="consts", bufs=1))
        wprep = ctx.enter_context(tc.tile_pool(name="wprep", bufs=2))
        xraw_p = ctx.enter_context(tc.tile_pool(name="xraw", bufs=xraw_bufs))
        ysb_p = ctx.enter_context(tc.tile_pool(name="ysb", bufs=ysb_bufs))
        conv_psum_bufs = 8 if prep_dve_t else (8 - psum_w_bufs if split_psum else 8)
        psum_p = ctx.enter_context(
            tc.tile_pool(name="psum", bufs=conv_psum_bufs, space="PSUM")
        )
        if split_psum and not prep_dve_t:
            psum_w = ctx.enter_context(
                tc.tile_pool(name="psumw", bufs=psum_w_bufs, space="PSUM")
            )
        else:
            psum_w = psum_p

        identity = consts.tile([P, P], bf16)
        make_identity(nc, identity)

        # weight slots: [ci(part), j=ci_tile, khw, co] lhsT + per-co scale
        wslots = []
        for s in range(n_slots):
            wT = consts.tile([P, CI_T, KH * KW, C], mm_dt, name=f"wT{s}")
            scale_sb = consts.tile([P, CO_T], f32, name=f"scale{s}")
            wslots.append((wT, scale_sb))

        # fixed activation tiles, one per image: [ci(part), j, padded plane]
        xtiles = [
            consts.tile([P, CI_T, PLANE], mm_dt, name=f"xt{i}") for i in range(N_IMG)
        ]
        for t in xtiles:
            nc.gpsimd.memset(t, 0.0)

        # ---------------- weight prep ----------------
        def weight_prep_phase1(slot):
            """DMA + reductions + sign: SP/DVE/ACT work, overlaps PE."""
            wT, scale_sb = wslots[slot]
            outs = []
            for co_t in range(CO_T):
                o0 = co_t * P
                w_sb = wprep.tile([P, C, KH * KW], f32, tag="w_sb",
                                  name=f"w_sb_{slot}_{co_t}")
                nc.sync.dma_start(out=w_sb, in_=w_flat[o0 : o0 + P])

                # two-stage mean over (i, khw): sums of 9, then sum of 256
                s1 = wprep.tile([P, C], f32, tag="s1", name=f"s1_{slot}_{co_t}")
                nc.vector.reduce_sum(out=s1, in_=w_sb, axis=mybir.AxisListType.X)
                s2 = wprep.tile([P, 1], f32, tag="s2", name=f"s2_{slot}_{co_t}")
                nc.vector.reduce_sum(out=s2, in_=s1, axis=mybir.AxisListType.X)
                mean = wprep.tile([P, 1], f32, tag="mean",
                                  name=f"mean_{slot}_{co_t}")
                nc.scalar.mul(out=mean, in_=s2, mul=1.0 / KFLAT)

                bw = wprep.tile([P, C, KH * KW], f32, tag="bw",
                                name=f"bw_{slot}_{co_t}")
                nc.vector.tensor_scalar_sub(out=bw, in0=w_sb, scalar1=mean)

                a1 = wprep.tile([P, C], f32, tag="a1", name=f"a1_{slot}_{co_t}")
                nc.vector.tensor_reduce(
                    out=a1,
                    in_=bw,
                    axis=mybir.AxisListType.X,
                    op=mybir.AluOpType.add,
                    apply_absolute_value=True,
                )
                a2 = wprep.tile([P, 1], f32, tag="a2", name=f"a2_{slot}_{co_t}")
                nc.vector.reduce_sum(out=a2, in_=a1, axis=mybir.AxisListType.X)
                nc.scalar.mul(
                    out=scale_sb[:, co_t : co_t + 1], in_=a2, mul=1.0 / KFLAT
                )

                # sign(bw), written permuted to [o(part), khw, i]; fp8 when
                # the DVE-transpose path consumes it (no convert step later)
                wsign = wprep.tile([P, KH * KW, C],
                                   fp8 if prep_dve_t else bf16, tag="wsign",
                                   name=f"wsign_{slot}_{co_t}")
                nc.scalar.sign(out=wsign.rearrange("p khw i -> p i khw"), in_=bw)
                outs.append(wsign)
            return outs

        def weight_prep_phase2(slot, wsigns):
            """Transpose wsign [o, khw, i] -> wT [i, j, khw, o].

            prep_dve_t: DVE stream-transpose (32x32 blocks in place) then 32
            small SBUF->SBUF DMAs on the ACT queue permute the block grid --
            zero PE involvement, so prep never extends the PE stream.
              tmp[32a+r, khw, 32m+c] = wsign[32a+c, khw, 32m+r]
              wT[32b+r, j, khw, o0+32a+c] = tmp[32a+r, khw, 128j+32b+c]
            Fallback: 36 PE transposes via identity matmuls + DVE copies.
            """
            wT, _ = wslots[slot]
            for co_t in range(CO_T):
                o0 = co_t * P
                wsign = wsigns[co_t]
                if prep_dve_t:
                    tmp = wprep.tile([P, KH * KW, C], fp8, tag="wtmp",
                                     name=f"wtmp_{slot}_{co_t}")
                    nc.vector.transpose(out=tmp, in_=wsign)
                    for a in range(P // 32):
                        src = tmp[32 * a : 32 * a + 32].rearrange(
                            "r khw (j b c) -> r khw j b c", j=CI_T, c=32
                        )
                        for b in range(P // 32):
                            nc.scalar.dma_start(
                                out=wT[
                                    32 * b : 32 * b + 32, :, :,
                                    o0 + 32 * a : o0 + 32 * a + 32,
                                ].rearrange("r j khw c -> r khw j c"),
                                in_=src[:, :, :, b],
                            )
                    continue
                # PE path: pair `prep_t_pair` transposes per PSUM tile; the
                # PSUM->wT copies alternate DVE/Pool so neither engine's
                # service rate gates the transpose cadence (and the conv
                # drains on DVE don't stall the prep ring).
                units = []  # (ci_t, khw) in PE emission order
                for ci_t in range(CI_T):
                    for khw in range(KH * KW):
                        units.append((ci_t, khw))
                for g0 in range(0, len(units), prep_t_pair):
                    grp = units[g0 : g0 + prep_t_pair]
                    pt = psum_w.tile(
                        [P, len(grp), P], bf16, tag="tp",
                        name=f"tp_{slot}_{co_t}_{g0}",
                    )
                    for gi, (ci_t, khw) in enumerate(grp):
                        nc.tensor.transpose(
                            pt[:, gi], wsign[:, khw, ci_t * P : ci_t * P + P],
                            identity,
                        )
                    # Pool cannot read PSUM; alternate DVE/ACT instead
                    use_act = prep_copy_split and (g0 // prep_t_pair) % 2
                    for gi, (ci_t, khw) in enumerate(grp):
                        dst = wT[:, ci_t, khw, o0 : o0 + P]
                        if use_act:
                            nc.scalar.copy(out=dst, in_=pt[:, gi])
                        else:
                            nc.vector.tensor_copy(out=dst, in_=pt[:, gi])

        def weight_prep(slot):
            weight_prep_phase2(slot, weight_prep_phase1(slot))

        # ---------------- conv ----------------
        dma_eng = nc.gpsimd if yout_gpsimd else nc.sync

        def conv_iter(k, tag):
            """One logical iteration: 4 images through the conv."""
            wT, scale_sb = wslots[k % n_slots]
            for img in range(N_IMG):
                xt = xtiles[img]
                if not skip_in_dma:
                    xr = xraw_p.tile([P, CI_T, H, W], f32, tag="xr",
                                     name=f"xr_{tag}_{img}")
                    nc.sync.dma_start(
                        out=xr,
                        in_=x_d[img].rearrange("(j c) h w -> c j h w", j=CI_T),
                    )
                    interior = xt[:, :, : HP * HP].rearrange(
                        "p j (h w) -> p j h w", w=HP
                    )[:, :, 1 : HP - 1, 1 : HP - 1]
                    if not skip_sign:
                        nc.scalar.sign(out=interior, in_=xr)

                xt_v = xt[:, :, : HP * HP].rearrange("p j (h w) -> p j h w", w=HP)
                for co_t in range(CO_T):
                    o0 = co_t * P
                    drains_on = not (skip_drain or skip_mm)
                    yimg = (
                        ysb_p.tile([P, H * W], y_dt, tag="ysb",
                                   name=f"yimg_{tag}_{img}_{co_t}")
                        if drains_on
                        else None
                    )
                    for b in range(NB):
                        ps = (
                            psum_p.tile(
                                [P, BLK, W], f32, tag="acc",
                                name=f"ps_{tag}_{img}_{co_t}_{b}",
                            )
                            if not skip_mm
                            else None
                        )
                        if not skip_mm:
                            if conv_bf16:
                                for ci_t in range(CI_T):
                                    for khw in range(KH * KW):
                                        kh, kw = divmod(khw, KW)
                                        nc.tensor.matmul(
                                            ps,
                                            wT[:, ci_t, khw, o0 : o0 + P],
                                            xt_v[
                                                :, ci_t,
                                                b * BLK + kh : b * BLK + kh + BLK,
                                                kw : kw + W,
                                            ],
                                            start=(ci_t == 0 and khw == 0),
                                            stop=(
                                                ci_t == CI_T - 1
                                                and khw == KH * KW - 1
                                            ),
                                        )
                            else:
                                for khw in range(KH * KW):
                                    kh, kw = divmod(khw, KW)
                                    nc.tensor.matmul(
                                        ps,
                                        wT[:, :, khw, o0 : o0 + P],
                                        xt_v[
                                            :, :,
                                            b * BLK + kh : b * BLK + kh + BLK,
                                            kw : kw + W,
                                        ],
                                        start=(khw == 0),
                                        stop=(khw == KH * KW - 1),
                                        perf_mode=mybir.MatmulPerfMode.DoubleRow,
                                    )
                        if drains_on:
                            ysb = yimg[:, b * NPIX_B : (b + 1) * NPIX_B]
                            if drain_act:
                                nc.scalar.mul(
                                    out=ysb, in_=ps,
                                    mul=scale_sb[:, co_t : co_t + 1],
                                )
                            else:
                                nc.vector.tensor_scalar_mul(
                                    out=ysb, in0=ps,
                                    scalar1=scale_sb[:, co_t : co_t + 1],
                                )
                    if drains_on and not skip_out_dma:
                        dma_eng.dma_start(out=y_flat[img, o0 : o0 + P], in_=yimg)

        def body(tick_tag, base_k):
            for u in range(unroll):
                k = base_k + u
                if prep_in_loop:
                    if prep_pipelined:
                        # prep slot (k+1): phase1 early, transposes after conv
                        ws = weight_prep_phase1((k + 1) % n_slots)
                        conv_iter(k, f"{tick_tag}_{u}")
                        weight_prep_phase2((k + 1) % n_slots, ws)
                    else:
                        weight_prep(k % n_slots)
                        conv_iter(k, f"{tick_tag}_{u}")
                else:
                    conv_iter(k, f"{tick_tag}_{u}")

        if not prep_in_loop or prep_pipelined:
            weight_prep(0)  # prologue: slot 0 ready before the first tick

        hints = (mybir.EngineType.PE,) if hint_pe else ()
        loop_cm = (
            tc.For_i(0, ticks, 1, hint_engines=hints, staggered_reset=staggered)
            if ticks > 1
            else contextlib.nullcontext()
        )
        with loop_cm:
            body("t", 0)

    nc.compile()
    return nc


_CACHED_NC = None


def kernel(x: np.ndarray, weight: np.ndarray) -> np.ndarray:
    global _CACHED_NC
    from concourse.bass_utils import run_bass_kernel_spmd

    if _CACHED_NC is None:
        _CACHED_NC = build_module(loop=1, prep_in_loop=False)
    nc = _CACHED_NC

    x = np.ascontiguousarray(x, dtype=np.float32)
    weight = np.ascontiguousarray(weight, dtype=np.float32)
    in_maps = [
        {"x": x[c * N_IMG : (c + 1) * N_IMG], "weight": weight}
        for c in range(N_CORES)
    ]
    res = run_bass_kernel_spmd(nc, in_maps, core_ids=list(range(N_CORES)))
    return np.concatenate(
        [r["y"].astype(np.float32) for r in res.results], axis=0
    )
